# revision 50
# baseline (speedup 1.0000x reference)
"""Trainium2 Bass kernel for BlockDiagMNIST MLP — v5 (16-way L2 packs).

Reference computation (all fp32):
    h  = relu(x @ W1.T + b1)          x:[B,784], W1:[4096,784]    -> [B,4096]
    yb = blockdiag(h, Wb)             Wb:[128,32,32] (h2[b, 32n+o] = sum_k h[b,32n+k] Wb[n,o,k])
    h2 = relu(yb + bb)
    out = h2 @ W3.T + b3              W3:[10,4096]                -> [B,10]

Data-parallel over batch (4096 rows/core on 8 cores), weights replicated,
bf16 matmuls, hidden-on-partitions layout (hT = W1 @ x.T per 512-col window).

PE-array packing (the PE is 16x 32x32 sub-arrays addressed by tile_position):
 - L1 leftover (features 768..783): 4 concurrent K=16 matmuls at row strips.
   Emitted FIRST in each group (start=True) so each m-tile's PSUM chain
   completes at its 6th K-chunk and evacuates early (allows ps1 bufs=4).
 - L2: all 16 diagonal blocks of a 4-m-tile group in ONE 16-way pack:
   tile (i, j) = Wb[(4g+i)*4+j].T at position (row 32j, col 32i), writing
   psum bank jj at partitions 32i.  Bank jj = h2-chunk (g, jj) with hidden
   permutation absorbed host-side into W3T/bb.
 - L3: chunk (g, jj) as an M=32 matmul on col strip jj -> 4 concurrent per
   group into the partition-quarters of one transient bank, which the DVE
   accumulates into an SBUF fp32 accumulator; a selection-matrix matmul
   folds the quarters at window end.

PSUM budget: ps1 = 4 banks (L1 chains), px = 4 banks shared by the L2 pack
(4 transient tiles) and the L3 quad (1 transient tile, timed apart).
"""

import numpy as np
import ml_dtypes

B = 32768
IN_DIM = 784
HIDDEN = 4096
BLOCK = 32
NUM_BLOCKS = 128
OUT_DIM = 10
NCORES = 8
BC = B // NCORES
WN = 512
K1 = 6
KL = 16
NM = HIDDEN // 128        # 32 m-tiles per window
NGW = NM // 4             # 8 groups per window

BF16 = ml_dtypes.bfloat16

_PROGRAM_CACHE = {}


def _build_program(bc=BC):
    import concourse.mybir as mybir
    import concourse.tile as tile
    from concourse import bacc

    nw = bc // WN
    f32, bf16 = mybir.dt.float32, mybir.dt.bfloat16

    nc = bacc.Bacc("TRN2", target_bir_lowering=False, debug=False)

    # x / W1 main blocks are pre-packed host-side into the exact per-DMA tile
    # layout [block, 128, KH, WN] so every tile DMA reads 3 KB contiguous per
    # partition (vs 3x 1KB strided rows) -- bigger bursts, fewer descriptors.
    KH = K1 // 2
    nw = bc // WN
    xT = nc.dram_tensor("xT", [nw * 2 * 128, KH * WN], bf16, kind="ExternalInput").ap()
    # leftover rows stored once in HBM; replicated to partition strips
    # 0/32/64/96 on-chip via 4 strip DMAs (saves ~0.9 MB of DMA traffic)
    xL = nc.dram_tensor("xL", [KL, bc], bf16, kind="ExternalInput").ap()
    w1t = nc.dram_tensor("W1T", [(NM // 4) * 2 * 128, KH * WN], bf16, kind="ExternalInput").ap()
    w1l = nc.dram_tensor("W1L", [KL, HIDDEN], bf16, kind="ExternalInput").ap()
    # L2 blocks: group g, sub-tile (i, j) = Wb[(4g+i)*4+j].T at
    # partitions 32j, cols 128g+32i
    wbt = nc.dram_tensor("Wbt", [128, NGW * 128], bf16, kind="ExternalInput").ap()
    # L3 chunk (g, jj) = [128, 32] lhsT at cols (4g+jj)*32 (cols 0..9 used),
    # rows permuted to match the L2 bank layout
    w3t = nc.dram_tensor("W3T", [128, NM * 32], bf16, kind="ExternalInput").ap()
    sel = nc.dram_tensor("SEL", [128, 32], bf16, kind="ExternalInput").ap()
    bcat = nc.dram_tensor("bcat", [128, 2 * NM + 1], f32, kind="ExternalInput").ap()
    outT = nc.dram_tensor("outT", [OUT_DIM, bc], f32, kind="ExternalOutput").ap()

    Relu = mybir.ActivationFunctionType.Relu
    Copy = mybir.ActivationFunctionType.Copy
    Add = mybir.AluOpType.add
    Max = mybir.AluOpType.max

    MB = 4
    NJ = NM // MB

    with tile.TileContext(nc) as tc:
        with (
            tc.tile_pool(name="const", bufs=1) as cpool,
            tc.tile_pool(name="xin", bufs=3) as xpool,
            tc.tile_pool(name="hbuf", bufs=10) as hpool,
            tc.tile_pool(name="h2buf", bufs=10) as h2pool,
            tc.tile_pool(name="accbuf", bufs=2) as accpool,
            tc.tile_pool(name="qbuf", bufs=2) as qpool,
            tc.tile_pool(name="obuf", bufs=2) as opool,
            tc.tile_pool(name="ps1", bufs=4, space="PSUM") as ps1,
            tc.tile_pool(name="px", bufs=4, space="PSUM") as px,
        ):
            bc_sb = cpool.tile([128, 2 * NM + 1], f32)
            nc.sync.dma_start(bc_sb[:], bcat)
            b1_sb = bc_sb[:, 0:NM]
            bb_sb = bc_sb[:, NM:2 * NM]
            b3_sb = bc_sb[0:OUT_DIM, 2 * NM:2 * NM + 1]

            # HAM warmup (~3.5us of junk matmuls on the early bias tile).
            pw = px.tile([65, 65], f32, tag="px", name="pwarm")
            for _ in range(14):
                nc.tensor.matmul(
                    pw[:], bc_sb[:, 0:2 * NM + 1], bc_sb[:, 0:2 * NM + 1],
                    start=True, stop=True,
                )

            xT_r = xT.rearrange("(b p) (k c) -> b p k c", p=128, k=KH)
            w1t_r = w1t.rearrange("(b p) (k c) -> b p k c", p=128, k=KH)

            def load_xl(w, tl):
                for j in range(4):
                    nc.sync.dma_start(tl[32 * j:32 * j + KL, :],
                                      xL[:, w * WN:(w + 1) * WN])

            def load_xt(w):
                tl = xpool.tile([128, WN], bf16, tag="xl", name=f"xl_{w}")
                if w > 0:
                    load_xl(w, tl)
                ta = xpool.tile([128, KH, WN], bf16, tag="xta", name=f"xta_{w}")
                nc.sync.dma_start(ta[:], xT_r[2 * w])
                tb = xpool.tile([128, K1 - KH, WN], bf16, tag="xtb", name=f"xtb_{w}")
                nc.sync.dma_start(tb[:], xT_r[2 * w + 1])
                return (ta, tb), tl

            # Window-0 leftover x + group-0's W1-leftover chunk land FIRST:
            # the group-0 leftover quad is the first real matmul after the
            # HAM warmup.  w1l/wbt/w3t are loaded as per-group chunk tiles
            # interleaved with the W1T blocks, so no later-needed bytes sit
            # ahead of earlier-needed ones on the DMA queues.
            xl0 = xpool.tile([128, WN], bf16, tag="xl", name="xl_0")
            load_xl(0, xl0)
            w1l_t = [cpool.tile([128, WN], bf16, name=f"w1l_{g}")
                     for g in range(NGW)]

            def load_w1l(g):
                for j in range(4):
                    nc.sync.dma_start(w1l_t[g][32 * j:32 * j + KL, :],
                                      w1l[:, g * WN:(g + 1) * WN])

            load_w1l(0)

            xts = {}

            def load_x0():
                ta = xpool.tile([128, KH, WN], bf16, tag="xta", name="xta_0")
                nc.sync.dma_start(ta[:], xT_r[0])
                tb = xpool.tile([128, K1 - KH, WN], bf16, tag="xtb", name="xtb_0")
                nc.sync.dma_start(tb[:], xT_r[1])
                return (ta, tb), xl0

            xts[0] = load_x0()

            w1t_t = [None] * NJ
            wbt_t = [None] * NGW
            w3t_t = [None] * NGW
            for j in range(NJ):
                if j > 0:
                    load_w1l(j)
                ta = cpool.tile([128, KH, MB * 128], bf16, name=f"w1ta_{j}")
                nc.sync.dma_start(ta[:], w1t_r[2 * j])
                tb = cpool.tile([128, K1 - KH, MB * 128], bf16, name=f"w1tb_{j}")
                nc.sync.dma_start(tb[:], w1t_r[2 * j + 1])
                w1t_t[j] = (ta, tb)
                wbt_t[j] = cpool.tile([128, 128], bf16, name=f"wbt_{j}")
                nc.sync.dma_start(wbt_t[j][:], wbt[:, j * 128:(j + 1) * 128])
                w3t_t[j] = cpool.tile([128, 128], bf16, name=f"w3t_{j}")
                nc.sync.dma_start(w3t_t[j][:], w3t[:, j * 128:(j + 1) * 128])
                if j == 0:
                    sel_sb = cpool.tile([128, 32], bf16)
                    nc.sync.dma_start(sel_sb[:], sel)

            NGTOT = nw * NGW
            hs = {}     # (w, g, i) -> h tile
            h2s = {}    # (w, g, jj) -> h2 tile (bank jj of group g's pack)
            accs = {}   # w -> sbuf fp32 accumulator / final bf16 q tile

            def emit_l1_leftover(G):
                """Leftover quad first (start=True) so each m-tile's PSUM
                chain completes at its 6th K-chunk and evacuates early."""
                w, g = divmod(G, NGW)
                if g == 0 and w not in xts:
                    xts[w] = load_xt(w)
                xt, xl = xts[w]
                p1s = []
                for j in range(4):
                    m = 4 * g + j
                    p1 = ps1.tile([128, WN], f32, tag="p1", name=f"p1_{G}_{j}")
                    nc.tensor.matmul(
                        p1[:],
                        w1l_t[g][32 * j:32 * j + KL, j * 128:(j + 1) * 128],
                        xl[32 * j:32 * j + KL, :],
                        start=True,
                        stop=False,
                        tile_position=(32 * j, 0),
                    )
                    p1s.append(p1)
                return p1s

            def emit_l1_chunks(G, p1s, js):
                w, g = divmod(G, NGW)
                xt, _ = xts[w]
                KH = K1 // 2
                for j in js:
                    m = 4 * g + j
                    for k in range(K1):
                        hi = k >= KH
                        nc.tensor.matmul(
                            p1s[j][:],
                            w1t_t[m // MB][hi][:, k - KH * hi, (m % MB) * 128:(m % MB + 1) * 128],
                            xt[hi][:, k - KH * hi, :],
                            start=False,
                            stop=(k == K1 - 1),
                        )
                    h = hpool.tile([128, WN], bf16, tag="h", name=f"h_{G}_{j}")
                    nc.scalar.activation(h[:], p1s[j][:], Relu,
                                         bias=b1_sb[:, m:m + 1])
                    hs[(w, g, j)] = h

            def emit_l2_group(G):
                """All 16 diagonal blocks of the group as ONE 16-way pack."""
                w, g = divmod(G, NGW)
                pb = [px.tile([128, WN], f32, tag="px", name=f"p2_{G}_{jj}")
                      for jj in range(4)]
                for jj in range(4):
                    for i in range(4):
                        nc.tensor.matmul(
                            pb[jj][32 * i:32 * i + 32, :],
                            wbt_t[g][32 * jj:32 * jj + 32,
                                     32 * i:32 * i + 32],
                            hs[(w, g, i)][32 * jj:32 * jj + 32, :],
                            start=True,
                            stop=True,
                            tile_position=(32 * jj, 32 * i),
                        )
                # Evacuations split across ACT and DVE so the chain gating the
                # next L3 quad is ~2 ops deep per engine instead of 4.
                for jj in range(4):
                    h2 = h2pool.tile([128, WN], bf16, tag="h2", name=f"h2_{G}_{jj}")
                    if jj < 2:
                        nc.scalar.activation(h2[:], pb[jj][:], Relu,
                                             bias=bb_sb[:, 4 * g + jj:4 * g + jj + 1])
                    else:
                        nc.vector.tensor_scalar(
                            h2[:], pb[jj][:],
                            bb_sb[:, 4 * g + jj:4 * g + jj + 1], 0.0, Add, Max)
                    h2s[(w, g, jj)] = h2
                for i in range(4):
                    hs.pop((w, g, i))

            def emit_l3_group(G):
                """4 concurrent col-strip matmuls into the quarters of one
                transient bank; DVE accumulates into an SBUF fp32 tile."""
                w, g = divmod(G, NGW)
                pq = px.tile([128, WN], f32, tag="px", name=f"pq_{G}")
                for jj in range(4):
                    nc.tensor.matmul(
                        pq[32 * jj:32 * jj + 32, :],
                        w3t_t[g][:, jj * 32:(jj + 1) * 32],
                        h2s.pop((w, g, jj))[:],
                        start=True,
                        stop=True,
                        skip_group_check=True,
                        tile_position=(0, 32 * jj),
                    )
                if g == 0:
                    acc = accpool.tile([128, WN], f32, tag="acc", name=f"acc_{w}")
                    nc.vector.tensor_copy(acc[:], pq[:])
                    accs[w] = acc
                elif g < NGW - 1:
                    acc = accs[w]
                    nc.vector.tensor_tensor(acc[:], acc[:], pq[:], Add)
                else:
                    q = qpool.tile([128, WN], bf16, tag="q", name=f"q_{w}")
                    nc.vector.tensor_tensor(q[:], accs.pop(w)[:], pq[:], Add)
                    rs = px.tile([32, WN], f32, tag="px", name=f"rs_{w}")
                    nc.tensor.matmul(rs[:], sel_sb[:], q[:], start=True, stop=True)
                    ot = opool.tile([OUT_DIM, WN], f32, tag="ot", name=f"ot_{w}")
                    nc.vector.tensor_scalar_add(ot[:], rs[0:OUT_DIM, :], b3_sb[:])
                    nc.sync.dma_start(outT[:, w * WN:(w + 1) * WN], ot[:])

            # Per-iteration emission order: the L3(G-2) quad sits between the
            # two L1 chunk halves, ~2.6us after the L2(G-2) pack whose DVE
            # evacuations produce its h2 inputs — so the DVE chain is done by
            # the time the PE reaches the quad (no head-of-line stall).
            for G in range(NGTOT + 2):
                w, g = divmod(G, NGW)
                if G < NGTOT and g == NGW - 2 and w + 1 < nw:
                    xts[w + 1] = load_xt(w + 1)   # prefetch next window's x
                if G < NGTOT:
                    p1s = emit_l1_leftover(G)
                    emit_l1_chunks(G, p1s, (0, 1, 2))
                if G >= 2:
                    emit_l3_group(G - 2)
                if G < NGTOT:
                    emit_l1_chunks(G, p1s, (3,))
                if 1 <= G <= NGTOT:
                    emit_l2_group(G - 1)

    nc.compile()
    return nc


def _get_program(bc=BC):
    if bc not in _PROGRAM_CACHE:
        _PROGRAM_CACHE[bc] = _build_program(bc)
    return _PROGRAM_CACHE[bc]


def _prep_weights(W1, b1, Wb, bb, W3, b3):
    W1 = np.asarray(W1, dtype=np.float32)
    Wb = np.asarray(Wb, dtype=np.float32)
    W3 = np.asarray(W3, dtype=np.float32)
    bb = np.asarray(bb, dtype=np.float32)

    # W1T packed as per-DMA tile blocks [NJ, 2, 128, KH, WN] (contiguous
    # per-partition bytes for each [128, KH, WN] tile DMA).
    NJ, KH = NM // 4, K1 // 2
    w1r = W1.T[:K1 * 128].astype(BF16).reshape(K1, 128, NJ, WN)
    W1TC = np.zeros((NJ, 2, 128, KH, WN), dtype=BF16)
    for j in range(NJ):
        for hf in range(2):
            W1TC[j, hf] = w1r[hf * KH:(hf + 1) * KH, :, j, :].transpose(1, 0, 2)
    W1T = W1TC.reshape(NJ * 2 * 128, KH * WN)
    W1L = np.ascontiguousarray(W1.T[K1 * 128:IN_DIM]).astype(BF16)

    # Wbt: group g, sub-tile (i, j) = Wb[(4g+i)*4+j].T at [32j, 128g+32i].
    Wbt = np.zeros((128, NGW * 128), dtype=BF16)
    for g in range(NGW):
        for i in range(4):
            for j in range(4):
                n = (4 * g + i) * 4 + j
                Wbt[32 * j:32 * j + 32,
                    g * 128 + 32 * i:g * 128 + 32 * i + 32] = Wb[n].T.astype(BF16)

    # W3T: chunk (g, jj) at cols (4g+jj)*32; row 32i+oo = hidden
    # ((4g+i)*4+jj)*32+oo (the L2 bank permutation).
    W3T = np.zeros((128, NM * 32), dtype=BF16)
    for g in range(NGW):
        for jj in range(4):
            c = 4 * g + jj
            for i in range(4):
                n = (4 * g + i) * 4 + jj
                W3T[32 * i:32 * i + 32, c * 32:c * 32 + OUT_DIM] = (
                    W3[:, n * 32:(n + 1) * 32].T.astype(BF16)
                )

    SEL = np.zeros((128, 32), dtype=BF16)
    for c in range(4):
        for o in range(OUT_DIM):
            SEL[32 * c + o, o] = 1

    bcat = np.zeros((128, 2 * NM + 1), dtype=np.float32)
    bcat[:, 0:NM] = np.asarray(b1, np.float32).reshape(NM, 128).T
    # bb column (4g+jj): partition 32i+o = bb[((4g+i)*4+jj)*32+o]
    for g in range(NGW):
        for jj in range(4):
            for i in range(4):
                n = (4 * g + i) * 4 + jj
                bcat[32 * i:32 * i + 32, NM + 4 * g + jj] = bb[n * 32:(n + 1) * 32]
    bcat[0:OUT_DIM, 2 * NM] = np.asarray(b3, np.float32)
    return dict(W1T=W1T, W1L=W1L, Wbt=Wbt, W3T=W3T, SEL=SEL, bcat=bcat)


def _prep_x_shard(x, c, ncores=NCORES, bc=BC):
    xs = np.asarray(x[c * bc:(c + 1) * bc], dtype=np.float32).T.astype(BF16)
    nw, KH = bc // WN, K1 // 2
    xr = xs[:K1 * 128].reshape(K1, 128, nw, WN)
    XTC = np.zeros((nw, 2, 128, KH, WN), dtype=BF16)
    for w in range(nw):
        for hf in range(2):
            XTC[w, hf] = xr[hf * KH:(hf + 1) * KH, :, w, :].transpose(1, 0, 2)
    xT = XTC.reshape(nw * 2 * 128, KH * WN)
    xLs = np.ascontiguousarray(xs[K1 * 128:IN_DIM])
    return xT, xLs


def run(x, W1, b1, Wb, bb, W3, b3, trace=False, tmpdir=None):
    from concourse.bass_utils import run_bass_kernel_spmd

    nc = _get_program()
    wmap = _prep_weights(W1, b1, Wb, bb, W3, b3)
    in_maps = []
    for c in range(NCORES):
        m = dict(wmap)
        m["xT"], m["xL"] = _prep_x_shard(np.asarray(x), c)
        in_maps.append(m)

    res = run_bass_kernel_spmd(
        nc, in_maps, core_ids=list(range(NCORES)), trace=trace, tmpdir=tmpdir
    )
    out = np.concatenate(
        [np.asarray(r["outT"]).T for r in res.results], axis=0
    ).astype(np.float32)
    return out, res


def kernel(x, W1, b1, Wb, bb, W3, b3):
    out, _ = run(x, W1, b1, Wb, bb, W3, b3, trace=False)
    return out
